# revision 14
# baseline (speedup 1.0000x reference)
"""Trainium2 Bass kernel for nn_Attention_4363686773373.

Sigmoid attention with magnitude-preserving (weight-normalized) projections.

Sharding: data-parallel over (batch, T-half) -> 8 shards on 8 NeuronCores.
Each core computes q for its 1024 tokens and k,v for the full 2048 tokens of
its batch (k/v recomputed on both cores; no collectives). Each core's xkv
rows are pre-ordered so its query tokens come first.

v2: fully fused single pipeline.  The scalar engine's 192 sigmoids
([128,1024] each, ~1us apiece = ~191us/core) are the hard floor; everything
else hides under them:
  - ONE ACT table set for the whole program (sigmoid_and_others: sigmoid +
    square + copy).  All sqrt/rsqrt are bit-trick + 2 Newton steps on
    DVE/GpSimd, so no table switches gate phase boundaries.
  - Emission order == execution schedule: q-weight rows -> x blocks ->
    q proj -> kv blocks trickle out while attention units consume them in
    key-superblocks sized 2,2,4,8; attn@v partials accumulate in one PSUM
    bank per (pair, t-half) and fold into an SBUF f32 accumulator, keeping
    PSUM within 8 banks while A, B, C phases run concurrently.
  - attn@v head pairs use explicit tile_position col-tiling (concurrent
    K=128/M=64 matmuls); score pairs use row-group concurrency (K=64 lhsT
    at partitions 0/64).
  - Norm squares/reduces/rsqrt run on GpSimd (otherwise idle), evictions
    and scale-muls on DVE, so ACT does nothing but sigmoid in steady state.
  - Phase C (per-token normalize + out-proj) interleaves into the last
    superblock: t-half 0 finalizes first, its out-proj runs under the
    remaining sigmoids; only t-half 1's tail is exposed.
"""

import math
from contextlib import ExitStack

import numpy as np

import concourse.bass as bass
import concourse.tile as tile
from concourse import bacc, mybir
from concourse.bass_utils import run_bass_kernel_spmd
from concourse.masks import make_identity

# Problem shapes (hardcoded per harness contract)
B, T, D, H = 4, 2048, 768, 12
HD = D // H  # 64
EPS = 1e-4
SIGMOID_GAIN = 1.8402
N_CORES = 8

F32 = mybir.dt.float32
BF16 = mybir.dt.bfloat16
I32 = mybir.dt.int32
AF = mybir.ActivationFunctionType
ALU = mybir.AluOpType
AX = mybir.AxisListType

RSQRT_MAGIC = 0x5F3759DF
AV_TILE_POS = True  # col-tile the two attn@v matmuls for concurrency


def _ensure_axon_hooks():
    """This image's antenv lacks axon_hooks; reconstruct it so trace=True
    (NTFF profiling) works instead of crashing on import."""
    try:
        import antenv.axon_hooks  # noqa: F401
        return
    except ImportError:
        pass
    import sys
    import types
    try:
        import antenv
    except ImportError:
        return
    mod = types.ModuleType("antenv.axon_hooks")
    _hook = [None]
    mod.set_axon_ntff_profile_hook = lambda h: _hook.__setitem__(0, h)
    mod.get_axon_ntff_profile_hook = lambda: _hook[0]
    sys.modules["antenv.axon_hooks"] = mod
    antenv.axon_hooks = mod
    try:
        from trn_agent_boot.trn_boot import _ntff_profile_via_ctypes
        mod.set_axon_ntff_profile_hook(
            _ntff_profile_via_ctypes('/opt/axon/libaxon_pjrt.so'))
    except Exception:
        pass


_ensure_axon_hooks()


def _chunks(total, maxn=512):
    out = []
    c0 = 0
    while c0 < total:
        cn = min(maxn, total - c0)
        out.append((c0, cn))
        c0 += cn
    return out


def _superblocks(nblocks):
    """Split range(nblocks) into groups sized ~n/8, n/8, n/4, n/2 so the
    attention sweep tracks kv production."""
    groups = []
    b0 = 0
    size = max(nblocks // 8, 1)
    while b0 < nblocks:
        size = min(size, nblocks - b0)
        groups.append(list(range(b0, b0 + size)))
        b0 += size
        if len(groups) >= 2:
            size *= 2
    return groups


def build_program(nc, tc, ctx, Tq, Tkv, Dm, Hn):
    keep = []  # keep tile free-closures alive
    stk = ExitStack()  # all pools; closed at end of build_program

    def _tile(shape, dtype, name):
        t, free = tc.tile(shape, dtype, name=name)
        keep.append(free)
        return t, free

    tc._ant_keepalive = keep
    P = 128
    HDl = 64
    assert Dm % P == 0 and Tq % P == 0 and Tkv % P == 0
    DT = Dm // P          # contraction d-tiles
    E3 = 3 * Dm
    PAIRS = Hn // 2
    assert PAIRS * P == Dm and Hn * HDl == Dm
    TBq = Tq // P
    TBkv = Tkv // P
    THW = min(512, Tq)    # t-chunk of one attention unit
    TH = Tq // THW
    QH = THW // P         # q blocks per t-half

    xkv = nc.dram_tensor("xkv", [Tkv, Dm], F32, kind="ExternalInput").ap()
    qkvw = nc.dram_tensor("qkvw", [E3, Dm], F32, kind="ExternalInput").ap()
    outw = nc.dram_tensor("outw", [Dm, Dm], F32, kind="ExternalInput").ap()
    y = nc.dram_tensor("y", [Tq, Dm], F32, kind="ExternalOutput").ap()

    # ---------------- DRAM scratch ----------------
    dpool = stk.enter_context(tc.tile_pool(name="dram", bufs=1, space="DRAM"))
    own_dram = dpool.tile([Dm, Dm], BF16, name="own_dram")
    kn_dram = dpool.tile([Tkv, Dm], BF16, name="kn_dram")
    qn_dram = dpool.tile([Tq, Dm], BF16, name="qn_dram")
    av_dram = dpool.tile([Dm, Tq], BF16, name="av_dram")  # [pairmajor-hd, t]

    # ---------------- persistent SBUF ----------------
    knT, _ = _tile([P, PAIRS * Tkv], BF16, "knT")    # [hd(2 heads), s]
    qnT, _ = _tile([P, PAIRS * Tq], BF16, "qnT")     # [hd(2 heads), t]
    vbig, _ = _tile([P, TBkv * Dm], BF16, "vbig")    # natural [s, e]
    ssx8, _ = _tile([P, max(TBq, 2)], F32, "ssx8")   # ||x_tok||^2 (bf16 x)
    risx, _ = _tile([P, max(TBq, 2)], F32, "risx")   # 1/ssx
    avacc, _ = _tile([P, PAIRS * Tq], BF16, "avacc")  # attn@v accumulator
    av16, _ = _tile([P, PAIRS * Tq], BF16, "av16")   # final attn@v (bf16)
    ident, _ = _tile([P, P], BF16, "ident")
    make_identity(nc, ident)

    # wnT/xkvT are freed mid-program (their SBUF is reused by ownT/avnT);
    # allocate them on the RIGHT side so their release doesn't violate the
    # left side's pool stack order.
    wx_stk = ExitStack()
    wnT_pool = wx_stk.enter_context(
        tc.tile_pool(name="wnTp", bufs=1, side="right"))
    xkvT_pool = wx_stk.enter_context(
        tc.tile_pool(name="xkvTp", bufs=1, side="right"))
    wnT = wnT_pool.tile([P, DT * E3], BF16, name="wnT")
    xkvT = xkvT_pool.tile([P, DT * Tkv], BF16, name="xkvT")

    # transient pools
    wstage = stk.enter_context(tc.tile_pool(name="wstage", bufs=3))
    xstage = stk.enter_context(tc.tile_pool(name="xstage", bufs=2))
    sqpool = stk.enter_context(tc.tile_pool(name="sqpool", bufs=3))
    nstage = stk.enter_context(tc.tile_pool(name="nstage", bufs=4))
    small = stk.enter_context(tc.tile_pool(name="small", bufs=8))
    smallg = stk.enter_context(tc.tile_pool(name="smallg", bufs=8))
    krawp = stk.enter_context(tc.tile_pool(name="krawp", bufs=2))
    attnp = stk.enter_context(tc.tile_pool(name="attnp", bufs=3))

    # PSUM: psA(2) + psS(4) + [psW(2 prefix) -> px(2)] = 8 banks
    psA = stk.enter_context(tc.tile_pool(name="psA", bufs=2, space="PSUM"))
    psS = stk.enter_context(tc.tile_pool(name="psS", bufs=2, space="PSUM"))
    psW_stk = ExitStack()
    psW = psW_stk.enter_context(tc.tile_pool(name="psW", bufs=2,
                                             space="PSUM"))

    # ---- rsqrt: bit trick + 2 Newton steps (no ACT tables involved) ----
    def rsqrt_chain(eng, pool, s_ap, n, final_scale=1.0, name="rs"):
        """SBUF f32 [P, n] tile holding final_scale/sqrt(s_ap)."""
        it = pool.tile([P, n], I32, name=f"{name}_i", tag=f"{name}i")
        # magic - (i>>1) == ((i>>1) ^ -1) + (magic+1); walrus forbids mixing
        # bitwise and arith ops in one tensor_scalar, so shift+xor then add.
        # The int ops run on DVE (Pool rejects bitwise tensor_scalar).
        nc.vector.tensor_scalar(it, s_ap.bitcast(I32), 1, -1,
                                op0=ALU.logical_shift_right,
                                op1=ALU.bitwise_xor)
        r0 = pool.tile([P, n], F32, name=f"{name}_r0", tag=f"{name}r0")
        nc.vector.tensor_scalar(r0.bitcast(I32), it, RSQRT_MAGIC + 1, None,
                                op0=ALU.add)
        t = pool.tile([P, n], F32, name=f"{name}_t", tag=f"{name}t")
        eng.tensor_tensor(t, r0, r0, op=ALU.mult)
        eng.tensor_tensor(t, t, s_ap, op=ALU.mult)
        eng.tensor_scalar(t, t, -0.5, 1.5, op0=ALU.mult, op1=ALU.add)
        r1 = pool.tile([P, n], F32, name=f"{name}_r1", tag=f"{name}r1")
        eng.tensor_tensor(r1, r0, t, op=ALU.mult)
        t2 = pool.tile([P, n], F32, name=f"{name}_t2", tag=f"{name}t")
        eng.tensor_tensor(t2, r1, r1, op=ALU.mult)
        eng.tensor_tensor(t2, t2, s_ap, op=ALU.mult)
        eng.tensor_scalar(t2, t2, -0.5 * final_scale, 1.5 * final_scale,
                          op0=ALU.mult, op1=ALU.add)
        r2 = pool.tile([P, n], F32, name=f"{name}_r2", tag=f"{name}r2")
        eng.tensor_tensor(r2, r1, t2, op=ALU.mult)
        return r2

    # ---- PE transpose batch: src [P, ndt*P] -> dst_big strided columns ----
    def pe_transpose_cols(pool, tag, src, dst_big, stride, base):
        ptw = pool.tile([P, DT * P], BF16, name="ptw", tag=tag)
        for dt in range(DT):
            nc.tensor.transpose(ptw[:, dt * P:(dt + 1) * P],
                                src[:, dt * P:(dt + 1) * P], ident)
        dst3 = dst_big.rearrange("p (dt s) -> p dt s", dt=DT)[
            :, :, base:base + P]
        nc.vector.tensor_copy(dst3, ptw.rearrange("p (dt s) -> p dt s",
                                                  dt=DT))

    # ---- W row-tile: load, normalize (ACT square + DVE rsqrt), transpose --
    def emit_w(we):
        wst = wstage.tile([P, Dm], F32, name="wst", tag="wst")
        nc.scalar.dma_start(wst, qkvw[we * P:(we + 1) * P, :])
        wsq = sqpool.tile([P, Dm], BF16, name="wsq", tag="sq")
        ssw = small.tile([P, 1], F32, name="ssw", tag="s1")
        nc.scalar.activation(wsq, wst, AF.Square, accum_out=ssw)
        rw = rsqrt_chain(nc.vector, small, ssw, 1, name="rw")
        wnb = nstage.tile([P, Dm], BF16, name="wnb", tag="nst")
        nc.vector.tensor_scalar_mul(wnb, wst, rw)
        pe_transpose_cols(psW, "ptw", wnb, wnT, E3, we * P)

    # ---- x block: load, cast, mag (q blocks), PE-transpose into xkvT ----
    def emit_x(ti):
        xst = xstage.tile([P, Dm], F32, name="xst", tag="xst")
        nc.sync.dma_start(xst, xkv[ti * P:(ti + 1) * P, :])
        xbf = nstage.tile([P, Dm], BF16, name="xbf", tag="nst")
        nc.vector.tensor_copy(xbf, xst)
        if ti < TBq:
            junk = sqpool.tile([P, Dm], BF16, name="junkx", tag="sq")
            nc.vector.scalar_tensor_tensor(
                junk, xbf, 1.0, xbf, op0=ALU.mult, op1=ALU.mult,
                accum_out=ssx8[:, ti:ti + 1])
        pe_transpose_cols(psW, "ptw", xbf, xkvT, Tkv, ti * P)

    # ---- qkv projection of block ti over output cols [col0, col0+ncols) --
    def proj_block(ti, col0, ncols, dst_evict):
        for (c0, cn) in _chunks(ncols, 512):
            ps = psA.tile([P, 512], F32, name="psa", tag="psa")
            for dt in range(DT):
                lhs = xkvT[:, dt * Tkv + ti * P: dt * Tkv + (ti + 1) * P]
                nc.tensor.matmul(
                    ps[:, 0:cn], lhsT=lhs,
                    rhs=wnT[:,
                            dt * E3 + col0 + c0: dt * E3 + col0 + c0 + cn],
                    start=(dt == 0), stop=(dt == DT - 1))
            dst_evict(c0, cn, ps[:, 0:cn])

    # ---- per-(token, head) cosine normalize of a raw q/k block ----
    def qk_normalize(kraw, is_k):
        sqk = sqpool.tile([P, Dm], BF16, name="sqk", tag="sq")
        nc.gpsimd.tensor_tensor(sqk, kraw, kraw, op=ALU.mult)
        ssk = smallg.tile([P, Hn], F32, name="ssk", tag="sh")
        nc.vector.tensor_reduce(ssk,
                                sqk.rearrange("p (h d) -> p h d", h=Hn),
                                axis=AX.X, op=ALU.add)
        # logits = (q*sqrt(HD)/|q|) . (k/|k|):  q carries the sqrt(HD)
        rk = rsqrt_chain(nc.gpsimd, smallg, ssk, Hn,
                         final_scale=(1.0 if is_k else math.sqrt(HDl)),
                         name="rk")
        knb = nstage.tile([P, Dm], BF16, name="knb", tag="nst")
        nc.vector.tensor_tensor(
            knb.rearrange("p (h d) -> p h d", h=Hn),
            kraw.rearrange("p (h d) -> p h d", h=Hn),
            rk.broadcast_to([P, Hn, HDl]),
            op=ALU.mult)
        return knb

    def emit_q(ti):
        qraw = krawp.tile([P, Dm], BF16, name="qraw", tag="kraw")

        def ev(c0, cn, ps):
            nc.vector.tensor_copy(qraw[:, c0:c0 + cn], ps)
        proj_block(ti, 0, Dm, ev)
        qnb = qk_normalize(qraw, False)
        nc.gpsimd.dma_start(qn_dram[ti * P:(ti + 1) * P, :], qnb)

    def emit_kv(ti):
        kraw = krawp.tile([P, Dm], BF16, name="kraw", tag="kraw")

        def ev(c0, cn, ps):
            if c0 + cn <= Dm:
                nc.vector.tensor_copy(kraw[:, c0:c0 + cn], ps)
            elif c0 >= Dm:
                nc.vector.tensor_copy(
                    vbig[:, ti * Dm + c0 - Dm: ti * Dm + c0 - Dm + cn], ps)
            else:
                kn = Dm - c0
                nc.vector.tensor_copy(kraw[:, c0:Dm], ps[:, 0:kn])
                nc.vector.tensor_copy(vbig[:, ti * Dm: ti * Dm + cn - kn],
                                      ps[:, kn:cn])
        proj_block(ti, Dm, 2 * Dm, ev)
        knb = qk_normalize(kraw, True)
        nc.gpsimd.dma_start(kn_dram[ti * P:(ti + 1) * P, :], knb)

    def emit_own(we):
        wst = wstage.tile([P, Dm], F32, name="wso", tag="wst")
        nc.scalar.dma_start(wst, outw[we * P:(we + 1) * P, :])
        junk = sqpool.tile([P, Dm], BF16, name="junkw", tag="sq")
        ssw = small.tile([P, 1], F32, name="sswo", tag="s1")
        nc.vector.scalar_tensor_tensor(
            junk, wst, 1.0, wst, op0=ALU.mult, op1=ALU.mult, accum_out=ssw)
        rw = rsqrt_chain(nc.vector, small, ssw, 1, name="rw")
        wnb = nstage.tile([P, Dm], BF16, name="wnbo", tag="nst")
        nc.vector.tensor_scalar_mul(wnb, wst, rw)
        nc.gpsimd.dma_start(own_dram[we * P:(we + 1) * P, :], wnb)

    def xbar_qn(th):
        h0, hn = th * THW, THW
        for pr in range(PAIRS):
            nc.sync.dma_start_transpose(
                qnT[:, pr * Tq + h0: pr * Tq + h0 + hn],
                qn_dram[h0:h0 + hn, pr * P:(pr + 1) * P])

    def xbar_kn(b0, nb):
        h0, hn = b0 * P, nb * P
        for pr in range(PAIRS):
            nc.sync.dma_start_transpose(
                knT[:, pr * Tkv + h0: pr * Tkv + h0 + hn],
                kn_dram[h0:h0 + hn, pr * P:(pr + 1) * P])

    # ================= prefix =================
    groups = _superblocks(TBkv)
    g0 = groups[0]
    for we in range(DT):                    # q rows of qkv_w
        emit_w(we)
    nx = max(QH, len(g0))
    for ti in range(nx):
        emit_x(ti)
    for ti in range(QH):                    # q blocks of t-half 0
        emit_q(ti)
    xbar_qn(0)
    for we in range(DT, 3 * DT):            # k,v rows of qkv_w
        emit_w(we)
    for sb in g0:
        emit_kv(sb)
    xbar_kn(g0[0], len(g0))
    for ti in range(nx, TBkv):              # remaining x blocks (PE
        emit_x(ti)                          # transposes need psW open)

    # ================= interleaved A-task schedule =================
    def atask(kind, arg):
        if kind == "q":
            emit_q(arg)
        elif kind == "xbarq":
            xbar_qn(arg)
        elif kind == "kv":
            emit_kv(arg)
        elif kind == "xbark":
            xbar_kn(*arg)
        elif kind == "own":
            emit_own(arg)

    # segments: (gi, th) sweeps for all groups; pre[s] = tasks that must be
    # fully emitted before segment s starts (spread during segment s-1).
    seg_keys = [(gi, th) for gi in range(len(groups)) for th in range(TH)]
    pre = {k: [] for k in seg_keys}
    pre["C"] = []
    if TH > 1:
        pre[(0, 1)] = ([("q", ti) for ti in range(QH, TBq)]
                       + [("xbarq", 1)])
    for gi in range(1, len(groups)):
        g = groups[gi]
        tasks = [("kv", sb) for sb in g]
        tasks += [("xbark", (g[0], len(g)))]
        pre[(gi, 0)] = tasks
    gilast0 = len(groups) - 1
    # out-proj weight prep must be emitted before phase C starts
    pre[(gilast0, 0)] = pre[(gilast0, 0)] + [("own", we) for we in range(DT)]

    # ================= phase B (+C interleave) =================
    psW_stk.close()
    px = stk.enter_context(tc.tile_pool(name="px", bufs=2, space="PSUM"))
    cstage = stk.enter_context(tc.tile_pool(name="cstage", bufs=2))
    ystage = stk.enter_context(tc.tile_pool(name="ystage", bufs=2))

    avpart = {}
    gilast = len(groups) - 1
    ownT = None
    avnT = None

    def finish_c_setup():
        # all wnT/xkvT users emitted; reuse their SBUF for ownT/avnT
        nonlocal ownT, avnT
        wx_stk.close()
        oa_pool = stk.enter_context(
            tc.tile_pool(name="oap", bufs=1, side="right"))
        ownT = oa_pool.tile([P, DT * Dm], BF16, name="ownT")
        avnT = oa_pool.tile([P, DT * Tq], BF16, name="avnT")
        for dt in range(DT):
            nc.sync.dma_start_transpose(
                ownT[:, dt * Dm:(dt + 1) * Dm],
                own_dram[:, dt * P:(dt + 1) * P])
        nc.vector.reciprocal(risx[:, 0:TBq], ssx8[:, 0:TBq])

    def emit_unit(gi, pr, th, sb):
        g = groups[gi]
        pss = psS.tile([P, 2 * THW], F32, name="pss", tag="pss")
        for a in (0, 1):
            r0 = a * HDl
            nc.tensor.matmul(
                pss[:, a * THW:(a + 1) * THW],
                lhsT=knT[r0:r0 + HDl,
                         pr * Tkv + sb * P: pr * Tkv + (sb + 1) * P],
                rhs=qnT[r0:r0 + HDl,
                        pr * Tq + th * THW: pr * Tq + (th + 1) * THW],
                start=True, stop=True)
        attn = attnp.tile([P, 2 * THW], BF16, name="attn", tag="attn")
        nc.scalar.activation(attn, pss, AF.Sigmoid)
        if sb == g[0]:
            avpart[(pr, th)] = px.tile([P, THW], F32, name="avp", tag="px")
        psav = avpart[(pr, th)]
        for a in (0, 1):
            r0 = a * HDl
            kw = dict(tile_position=(0, r0)) if AV_TILE_POS else {}
            nc.tensor.matmul(
                psav[r0:r0 + HDl, :],
                lhsT=vbig[:, sb * Dm + pr * P + r0:
                          sb * Dm + pr * P + r0 + HDl],
                rhs=attn[:, a * THW:(a + 1) * THW],
                start=(sb == g[0]), stop=(sb == g[-1]),
                skip_group_check=True, **kw)
        if sb == g[-1]:
            psav = avpart.pop((pr, th))
            acc = avacc[:, pr * Tq + th * THW: pr * Tq + (th + 1) * THW]
            fin = av16[:, pr * Tq + th * THW: pr * Tq + (th + 1) * THW]
            if gi == 0 and gi == gilast:
                nc.vector.tensor_copy(fin, psav)
            elif gi == 0:
                nc.vector.tensor_copy(acc, psav)
            elif gi == gilast:
                nc.vector.tensor_tensor(fin, acc, psav, op=ALU.add)
            else:
                nc.vector.tensor_tensor(acc, acc, psav, op=ALU.add)

    # ---- phase C for one t-half ----
    def emit_c_half(th):
        for pr in range(PAIRS):
            nc.gpsimd.dma_start(
                av_dram[pr * P:(pr + 1) * P, th * THW:(th + 1) * THW],
                av16[:, pr * Tq + th * THW: pr * Tq + (th + 1) * THW])
        for tb in range(th * QH, (th + 1) * QH):
            avnat = cstage.tile([P, Dm], BF16, name="avnat", tag="avnat")
            nc.sync.dma_start_transpose(avnat,
                                        av_dram[:, tb * P:(tb + 1) * P])
            sqa = sqpool.tile([P, Dm], BF16, name="sqa", tag="sq")
            nc.gpsimd.tensor_tensor(sqa, avnat, avnat, op=ALU.mult)
            ssa = smallg.tile([P, Hn], F32, name="ssa", tag="sh")
            nc.vector.tensor_reduce(
                ssa, sqa.rearrange("p (h d) -> p h d", h=Hn),
                axis=AX.X, op=ALU.add)
            spr = smallg.tile([P, Hn], F32, name="spr", tag="sh")
            nc.gpsimd.tensor_scalar_mul(spr, ssa, risx[:, tb:tb + 1])
            ga = rsqrt_chain(nc.gpsimd, smallg, spr, Hn,
                             final_scale=math.sqrt(HDl / Dm), name="rc")
            avn = cstage.tile([P, Dm], BF16, name="avn", tag="avn")
            nc.vector.tensor_tensor(
                avn.rearrange("p (h d) -> p h d", h=Hn),
                avnat.rearrange("p (h d) -> p h d", h=Hn),
                ga.broadcast_to([P, Hn, HDl]),
                op=ALU.mult)
            pe_transpose_cols(px, "px", avn, avnT, Tq, tb * P)
            # out-projection of this token block
            ysb = ystage.tile([P, Dm], F32, name="ysb", tag="ysb")
            for (c0, cn) in _chunks(Dm, 512):
                pso = psA.tile([P, 512], F32, name="pso", tag="psa")
                for dt in range(DT):
                    lhs = avnT[:, dt * Tq + tb * P: dt * Tq + (tb + 1) * P]
                    nc.tensor.matmul(
                        pso[:, 0:cn], lhsT=lhs,
                        rhs=ownT[:, dt * Dm + c0: dt * Dm + c0 + cn],
                        start=(dt == 0), stop=(dt == DT - 1))
                nc.vector.tensor_copy(ysb[:, c0:c0 + cn], pso[:, 0:cn])
            nc.gpsimd.dma_start(y[tb * P:(tb + 1) * P, :], ysb)

    # ---- emit all segments with interleaved A tasks ----
    for si, (gi, th) in enumerate(seg_keys):
        nxt = seg_keys[si + 1] if si + 1 < len(seg_keys) else "C"
        tasks = pre[nxt]
        units = [(gi, pr, th, sb) for pr in range(PAIRS)
                 for sb in groups[gi]]
        t_done = 0
        for ui, (ugi, pr, uth, sb) in enumerate(units):
            emit_unit(ugi, pr, uth, sb)
            want = ((ui + 1) * len(tasks) + len(units) - 1) // len(units)
            while t_done < want:
                atask(*tasks[t_done])
                t_done += 1
        if (gi, th) == (gilast, 0) and TH > 1:
            # all A work emitted; set up phase C inputs, overlap C(th=0)
            finish_c_setup()
            emit_c_half(0)

    if TH == 1:
        finish_c_setup()
        emit_c_half(0)
    else:
        emit_c_half(1)
    stk.close()


def make_nc(Tq=T // 2, Tkv=T, Dm=D, Hn=H):
    nc = bacc.Bacc("TRN2", target_bir_lowering=False, debug=False,
                   num_devices=N_CORES)
    with ExitStack() as ctx:
        with tile.TileContext(nc) as tc:
            build_program(nc, tc, ctx, Tq, Tkv, Dm, Hn)
    nc.compile()
    return nc


_CACHED_NC = None


def _get_nc():
    global _CACHED_NC
    if _CACHED_NC is None:
        _CACHED_NC = make_nc()
    return _CACHED_NC


def _shard_inputs(x, qkv_w, out_w):
    Tq = T // 2
    x = np.asarray(x, dtype=np.float32)
    qkv_w = np.ascontiguousarray(np.asarray(qkv_w, dtype=np.float32))
    out_w = np.ascontiguousarray(np.asarray(out_w, dtype=np.float32))
    in_maps = []
    for core in range(N_CORES):
        b, half = core // 2, core % 2
        own = x[b, half * Tq:(half + 1) * Tq]
        other = x[b, (1 - half) * Tq:(2 - half) * Tq]
        xkv = np.ascontiguousarray(np.concatenate([own, other], axis=0))
        in_maps.append({"xkv": xkv, "qkvw": qkv_w, "outw": out_w})
    return in_maps


def run(x, qkv_w, out_w, trace=False, trace_cores=None):
    nc = _get_nc()
    in_maps = _shard_inputs(x, qkv_w, out_w)
    res = run_bass_kernel_spmd(nc, in_maps, list(range(N_CORES)),
                               trace=trace, trace_cores=trace_cores)
    Tq = T // 2
    y = np.empty((B, T, D), np.float32)
    for core, r in enumerate(res.results):
        b, half = core // 2, core % 2
        y[b, half * Tq:(half + 1) * Tq] = r["y"]
    return y, res


def kernel(x, qkv_w, out_w):
    y, _ = run(x, qkv_w, out_w, trace=False)
    return y
